# revision 11
# baseline (speedup 1.0000x reference)
"""Self-contained Trainium2 Bass kernel for nn_AttnLayer_71382356460296.

Sharding: data-parallel over batch B (2) x sequence-parallel over query
chunks (4) => 8 cores. Each core computes its (batch, 1024-query chunk)
slice of the full layer: q-projection + RoPE, windowed GQA attention with
a top-left-aligned causal mask, and the output projection. No reduction
is needed across cores - the host just concatenates the 8 output chunks.

v2: bf16 matmul operands (fp32 PSUM accumulation), attention kept fully
in SBUF (no DRAM round-trip), causal column-mask applied via
row-concurrent bias matmuls (partitions 0:8 / 64:72), softmax
normalization via per-head reciprocal + gpsimd partition_broadcast, and
the output projection streams Wo against attention tiles used as the
stationary operand.
"""

import time

import numpy as np

import concourse.bacc as bacc
import concourse.mybir as mybir
import concourse.tile as tile
from concourse.bass_utils import run_bass_kernel_spmd

F32 = mybir.dt.float32
BF16 = mybir.dt.bfloat16
AF = mybir.ActivationFunctionType

FULL = dict(B=2, T=4096, D=2048, H=32, KV=8, DH=64, W=1024, BASE=10000.0)
BIGNEG = -1e30


def _derived(cfg):
    d = dict(cfg)
    d["CH"] = cfg["T"] // 4            # queries per core
    d["KB"] = cfg["W"] // 128          # 128-key blocks in window
    d["DT"] = cfg["D"] // 128          # contraction tiles for Wq
    d["NP"] = cfg["H"] // 2            # head pairs (= D/128 output tiles)
    d["NC"] = [(i, min(512, d["CH"] - i)) for i in range(0, d["CH"], 512)]
    d["OC"] = [(i, min(512, cfg["D"] - i)) for i in range(0, cfg["D"], 512)]
    assert d["NP"] * 128 == cfg["D"]
    return d


def build(cfg):
    c = _derived(cfg)
    CH, KB, DT, NP, KV, H = c["CH"], c["KB"], c["DT"], c["NP"], c["KV"], c["H"]
    hpkv = H // KV
    nc = bacc.Bacc("TRN2", target_bir_lowering=False, debug=False)

    xT = nc.dram_tensor("xT", [c["D"], CH], BF16, kind="ExternalInput")
    wqT = nc.dram_tensor("wqT", [c["D"], c["D"]], BF16, kind="ExternalInput")
    woT = nc.dram_tensor("woT", [c["D"], c["D"]], BF16, kind="ExternalInput")
    kT = nc.dram_tensor("kT", [64, KV, KB, 128], BF16, kind="ExternalInput")
    vaug0 = nc.dram_tensor("vaug0", [128, KV, KB, 65], BF16,
                           kind="ExternalInput")
    vaug1 = nc.dram_tensor("vaug1", [128, KV, KB, 128], BF16,
                           kind="ExternalInput")
    cosT = nc.dram_tensor("cosT", [128, CH], BF16, kind="ExternalInput")
    sinT = nc.dram_tensor("sinT", [128, CH], BF16, kind="ExternalInput")
    tri = nc.dram_tensor("tri", [128, 128], BF16, kind="ExternalInput")
    brow = nc.dram_tensor("brow", [KB, CH], BF16, kind="ExternalInput")
    sel = nc.dram_tensor("sel", [8, KB, 128], BF16, kind="ExternalInput")
    out = nc.dram_tensor("out", [CH, c["D"]], F32, kind="ExternalOutput")

    swap = [i ^ 1 for i in range(32)]

    with nc.allow_low_precision(reason="bf16 matmuls are intended"), \
         tile.TileContext(nc) as tc:
        with (
            tc.tile_pool(name="consts", bufs=1) as cp,
            tc.tile_pool(name="qt", bufs=1) as qtp,
            tc.tile_pool(name="at", bufs=1) as atp,
            tc.tile_pool(name="psbig", bufs=2, space="PSUM") as psb,
            tc.tile_pool(name="psav", bufs=1, space="PSUM") as psa,
        ):
            cos_sb = cp.tile([128, CH], BF16)
            nc.sync.dma_start(cos_sb[:], cosT[:])
            sin_sb = cp.tile([128, CH], BF16)
            nc.sync.dma_start(sin_sb[:], sinT[:])
            # wo lives at outer scope so its load overlaps phase B
            wo_sb = cp.tile([128, NP, c["D"]], BF16)
            qts = []
            ats = []

            # ---- Phase A: q = rope(x @ WqT) in transposed per-pair layout
            with (
                tc.tile_pool(name="xts", bufs=1) as xp,
                tc.tile_pool(name="wq", bufs=1) as wp,
                tc.tile_pool(name="rope", bufs=2) as rp,
            ):
                xts = xp.tile([128, DT, CH], BF16)
                nc.sync.dma_start(
                    xts[:], xT.rearrange("(kt p) f -> p kt f", p=128))
                wq_sb = wp.tile([128, DT, c["D"]], BF16)
                nc.sync.dma_start(
                    wq_sb[:], wqT.rearrange("(kt p) e -> p kt e", p=128))
                nc.sync.dma_start(
                    wo_sb[:], woT.rearrange("(m p) o -> p m o", p=128))
                for m in range(NP):
                    qp = psb.tile([128, CH], F32, tag="big")
                    for kt in range(DT):
                        for n0, nn in c["NC"]:
                            nc.tensor.matmul(
                                qp[:, n0:n0 + nn],
                                wq_sb[:, kt, 128 * m:128 * (m + 1)],
                                xts[:, kt, n0:n0 + nn],
                                start=(kt == 0), stop=(kt == DT - 1))
                    qs = rp.tile([128, CH], F32, tag="qs")
                    nc.vector.stream_shuffle(qs[:], qp[:], swap)
                    t1 = rp.tile([128, CH], F32, tag="t1")
                    nc.vector.tensor_mul(t1[:], qp[:], cos_sb[:])
                    t2 = rp.tile([128, CH], F32, tag="t2")
                    nc.gpsimd.tensor_mul(t2[:], qs[:], sin_sb[:])
                    qt = qtp.tile([128, CH], BF16, tag=f"qt{m}")
                    nc.gpsimd.tensor_add(qt[:], t1[:], t2[:])
                    qts.append(qt)

            # ---- Phase B: attention per head pair, scores transposed
            with (
                tc.tile_pool(name="kv", bufs=1) as kp,
                tc.tile_pool(name="expp", bufs=4) as ep,
                tc.tile_pool(name="rcp", bufs=2) as rcp,
            ):
                ktd_sb = kp.tile([128, KV, KB, 128], BF16)
                nc.sync.dma_start(ktd_sb[0:64], kT[:])
                nc.sync.dma_start(ktd_sb[64:128], kT[:])
                va0_sb = kp.tile([128, KV, KB, 65], BF16)
                nc.sync.dma_start(va0_sb[:], vaug0[:])
                va1_sb = kp.tile([128, KV, KB, 128], BF16)
                nc.sync.dma_start(va1_sb[:], vaug1[:])
                tri_sb = kp.tile([128, 128], BF16)
                nc.sync.dma_start(tri_sb[:], tri[:])
                br_sb = kp.tile([128, CH], BF16)
                nc.sync.dma_start(br_sb[0:KB], brow[:])
                nc.sync.dma_start(br_sb[64:64 + KB], brow[:])
                sel_sb = kp.tile([128, KB, 128], BF16)
                nc.sync.dma_start(sel_sb[0:8], sel[:])
                nc.sync.dma_start(sel_sb[64:72], sel[:])

                for m in range(NP):
                    kv = (2 * m) // hpkv
                    av0 = psa.tile([128, CH], F32, tag="av0")
                    av1 = psa.tile([128, CH], F32, tag="av1")
                    for kb in range(KB):
                        # both heads' QK back-to-back: disjoint PE row
                        # groups (0:64 / 64:128) execute concurrently
                        sps = []
                        for hh in range(2):
                            sp = psb.tile([128, CH], F32, tag="big")
                            lh = ktd_sb[64 * hh:64 * (hh + 1), kv, kb, :]
                            rh = qts[m][64 * hh:64 * (hh + 1)]
                            for n0, nn in c["NC"]:
                                mb = min(128 * kb, n0 + nn)
                                nc.tensor.matmul(
                                    sp[:, n0:n0 + nn], lh,
                                    rh[:, n0:n0 + nn],
                                    start=True, stop=not (mb > n0))
                            sps.append(sp)
                        # causal column mask: bias-row matmuls on
                        # row groups 0:8 / 64:72 (concurrent)
                        for hh in range(2):
                            for n0, nn in c["NC"]:
                                mb = min(128 * kb, n0 + nn)
                                if mb > n0:
                                    nc.tensor.matmul(
                                        sps[hh][:, n0:mb],
                                        sel_sb[64 * hh:64 * hh + 8, kb, :],
                                        br_sb[64 * hh:64 * hh + 8, n0:mb],
                                        start=False, stop=True)
                        ers = []
                        for hh in range(2):
                            er = ep.tile([128, CH], BF16, tag="er")
                            nc.scalar.activation(er[:], sps[hh][:], AF.Exp)
                            if 128 * (kb + 1) <= CH:
                                dsl = slice(128 * kb, 128 * (kb + 1))
                                nc.vector.tensor_mul(er[:, dsl], er[:, dsl],
                                                     tri_sb[:])
                            ers.append(er)
                        for n0, nn in c["NC"]:
                            nc.tensor.matmul(
                                av0[0:65, n0:n0 + nn],
                                va0_sb[:, kv, kb, :], ers[0][:, n0:n0 + nn],
                                start=(kb == 0), stop=(kb == KB - 1))
                        for n0, nn in c["NC"]:
                            nc.tensor.matmul(
                                av1[:, n0:n0 + nn],
                                va1_sb[:, kv, kb, :], ers[1][:, n0:n0 + nn],
                                start=(kb == 0), stop=(kb == KB - 1))
                    # normalize: den_h0 at av0 row 64, den_h1 at av1 row 0
                    rec = rcp.tile([128, CH], F32, tag="rec")
                    nc.vector.reciprocal_approx_fast(rec[64:65], av0[64:65])
                    nc.vector.reciprocal_approx_fast(rec[0:1], av1[0:1])
                    bcs = rcp.tile([128, CH], F32, tag="bcs")
                    nc.gpsimd.partition_broadcast(bcs[0:64], rec[64:65])
                    nc.gpsimd.partition_broadcast(bcs[64:128], rec[0:1])
                    at = atp.tile([128, CH], BF16, tag=f"at{m}")
                    nc.vector.tensor_mul(at[0:64], av0[0:64], bcs[0:64])
                    nc.vector.tensor_mul(at[64:128], av1[64:128],
                                         bcs[64:128])
                    ats.append(at)

            # ---- Phase C: out[q, o] = sum_m at_m.T @ woT_m
            with (
                tc.tile_pool(name="osb", bufs=3) as op_,
            ):
                MQ = CH // 128
                for o0, ow in c["OC"]:
                    for mq in range(MQ):
                        opx = psb.tile([128, 512], F32, tag="big")
                        for kq in range(NP):
                            nc.tensor.matmul(
                                opx[:, :ow],
                                ats[kq][:, 128 * mq:128 * (mq + 1)],
                                wo_sb[:, kq, o0:o0 + ow],
                                start=(kq == 0), stop=(kq == NP - 1))
                        osb = op_.tile([128, 512], F32, tag="os")
                        if mq % 2 == 0:
                            nc.scalar.copy(osb[:, :ow], opx[:, :ow])
                        else:
                            nc.vector.tensor_copy(osb[:, :ow], opx[:, :ow])
                        nc.sync.dma_start(
                            out[128 * mq:128 * (mq + 1), o0:o0 + ow],
                            osb[:, :ow])
    nc.compile()
    return nc


def host_inputs(cfg, x, k_cache, v_cache, Wq, Wo, core):
    import ml_dtypes
    c = _derived(cfg)
    CH, KB, KV, W, DH = c["CH"], c["KB"], c["KV"], c["W"], c["DH"]
    b, ch = core // 4, core % 4
    Tc = k_cache.shape[2]
    f32 = np.float32
    bf16 = ml_dtypes.bfloat16

    xT = np.ascontiguousarray(x[b, CH * ch:CH * (ch + 1), :].T).astype(bf16)
    wqT = (np.ascontiguousarray(Wq.T) * f32(1.0 / np.sqrt(DH))).astype(bf16)
    woT = np.ascontiguousarray(Wo.T).astype(bf16)
    kw = k_cache[b, :, Tc - W:, :]                      # (KV, W, DH)
    kT = np.ascontiguousarray(
        kw.reshape(KV, KB, 128, DH).transpose(3, 0, 1, 2)).astype(bf16)
    vw = v_cache[b, :, Tc - W:, :].reshape(KV, KB, 128, DH)
    vaug0 = np.ones((128, KV, KB, 65), f32)
    vaug0[:, :, :, :DH] = vw.transpose(2, 0, 1, 3)
    vaug1 = np.zeros((128, KV, KB, 128), f32)
    vaug1[:, :, :, 64:128] = vw.transpose(2, 0, 1, 3)
    vaug1[:, :, :, 0] = 1.0
    pos = (CH * ch + np.arange(CH)).astype(f32)
    inv = 1.0 / (cfg["BASE"] ** (np.arange(0, DH, 2, dtype=f32) / DH))
    r = np.arange(128)
    u = (r % 64) // 2
    ang = pos[None, :] * inv[u][:, None]                # (128, CH)
    cosT = np.cos(ang).astype(bf16)
    sinT = (np.sin(ang) * np.where(r % 2 == 0, -1.0, 1.0)[:, None]).astype(bf16)
    if ch == 0:
        tri = (np.arange(128)[:, None] <= np.arange(128)[None, :]).astype(f32)
        brow = np.zeros((KB, CH), f32)
        for kb in range(KB):
            brow[kb, :128 * kb] = BIGNEG
    else:
        tri = np.ones((128, 128), f32)
        brow = np.zeros((KB, CH), f32)
    sel = np.zeros((8, KB, 128), f32)
    for kb in range(KB):
        sel[kb, kb, :] = 1.0
    return {"xT": xT, "wqT": wqT, "woT": woT, "kT": kT,
            "vaug0": vaug0.astype(bf16), "vaug1": vaug1.astype(bf16),
            "cosT": cosT, "sinT": sinT, "tri": tri.astype(bf16),
            "brow": brow.astype(bf16), "sel": sel.astype(bf16)}


_NC_CACHE = {}


def run(cfg, x, k_cache, v_cache, Wq, Wo, trace=False):
    key = tuple(sorted((k, v) for k, v in cfg.items()))
    if key not in _NC_CACHE:
        _NC_CACHE[key] = build(cfg)
    nc = _NC_CACHE[key]
    in_maps = [host_inputs(cfg, x, k_cache, v_cache, Wq, Wo, c)
               for c in range(8)]
    res = None
    for attempt in range(3):
        try:
            res = run_bass_kernel_spmd(nc, in_maps, core_ids=list(range(8)),
                                       trace=trace)
            break
        except Exception:
            if attempt == 2:
                raise
            time.sleep(2.0)
    outs = [res.results[c]["out"] for c in range(8)]
    full = np.stack([np.concatenate(outs[0:4], axis=0),
                     np.concatenate(outs[4:8], axis=0)])
    return full, res


def kernel(x, k_cache, v_cache, Wq, Wo):
    full, _ = run(FULL, np.asarray(x), np.asarray(k_cache),
                  np.asarray(v_cache), np.asarray(Wq), np.asarray(Wo))
    return full.astype(np.float32)


# revision 44
# speedup vs baseline: 1.0392x; 1.0392x over previous
"""Self-contained Trainium2 Bass kernel for nn_AttnLayer_71382356460296.

Sharding: data-parallel over batch B (2) x sequence-parallel over query
chunks (4) => 8 cores. Each core computes its (batch, 1024-query chunk)
slice of the full layer: q-projection + RoPE, windowed GQA attention with
a top-left-aligned causal mask, and the output projection. No reduction
is needed across cores - the host just concatenates the 8 output chunks.

v2: bf16 matmul operands (fp32 PSUM accumulation), attention kept fully
in SBUF (no DRAM round-trip), causal column-mask applied via
row-concurrent bias matmuls (partitions 0:8 / 64:72), softmax
normalization via per-head reciprocal + gpsimd partition_broadcast, and
the output projection streams Wo against attention tiles used as the
stationary operand.
"""

import time

import numpy as np

import concourse.bacc as bacc
import concourse.mybir as mybir
import concourse.tile as tile
from concourse.bass_utils import run_bass_kernel_spmd

F32 = mybir.dt.float32
F32R = mybir.dt.float32r
BF16 = mybir.dt.bfloat16
AF = mybir.ActivationFunctionType

FULL = dict(B=2, T=4096, D=2048, H=32, KV=8, DH=64, W=1024, BASE=10000.0)
BIGNEG = -1e30


def _derived(cfg):
    d = dict(cfg)
    d["CH"] = cfg["T"] // 4            # queries per core
    d["KB"] = cfg["W"] // 128          # 128-key blocks in window
    d["DT"] = cfg["D"] // 128          # contraction tiles for Wq
    d["NP"] = cfg["H"] // 2            # head pairs (= D/128 output tiles)
    d["NC"] = [(i, min(512, d["CH"] - i)) for i in range(0, d["CH"], 512)]
    d["OC"] = [(i, min(512, cfg["D"] - i)) for i in range(0, cfg["D"], 512)]
    d["WC"] = cfg["D"] // 512
    assert d["NP"] * 128 == cfg["D"]
    return d


def build(cfg, dbg=False):
    c = _derived(cfg)
    CH, KB, DT, NP, KV, H = c["CH"], c["KB"], c["DT"], c["NP"], c["KV"], c["H"]
    hpkv = H // KV
    nc = bacc.Bacc("TRN2", target_bir_lowering=False, debug=False)

    # host-rearranged: xr[p, kt, f] = xT[kt*128+p, f]; wqr[p, kt, e] =
    # wqT[kt*128+p, e] (chunked on e); wor[p, oc, m, o] = woT[m*128+p,
    # oc*512+o] — every DMA is one contiguous run per partition.
    xr = nc.dram_tensor("xr", [128, DT, CH], BF16, kind="ExternalInput")
    wqr = nc.dram_tensor("wqr", [128, c["WC"], DT, 512], BF16,
                         kind="ExternalInput")
    wor = nc.dram_tensor("wor", [128, c["WC"], NP, 512], BF16,
                         kind="ExternalInput")
    kT = nc.dram_tensor("kT", [64, KV, KB, 128], BF16, kind="ExternalInput")
    vaug0 = nc.dram_tensor("vaug0", [128, KV, KB, 128], BF16,
                           kind="ExternalInput")
    vaug1 = nc.dram_tensor("vaug1", [128, KV, KB, 128], BF16,
                           kind="ExternalInput")
    cosT = nc.dram_tensor("cosT", [128, CH], BF16, kind="ExternalInput")
    sinT = nc.dram_tensor("sinT", [128, CH], BF16, kind="ExternalInput")
    tri = nc.dram_tensor("tri", [128, 128], BF16, kind="ExternalInput")
    brow = nc.dram_tensor("brow", [KB, CH], BF16, kind="ExternalInput")
    sel = nc.dram_tensor("sel", [KB, KB, 128], BF16, kind="ExternalInput")
    ebc = nc.dram_tensor("ebc", [2, 128], F32R, kind="ExternalInput")
    out = nc.dram_tensor("out", [CH, c["D"]], F32, kind="ExternalOutput")
    if dbg:
        qdbg = nc.dram_tensor("qdbg", [128, CH], BF16, kind="ExternalOutput")
        edbg = nc.dram_tensor("edbg", [128, CH], BF16, kind="ExternalOutput")
        e1dbg = nc.dram_tensor("e1dbg", [128, CH], BF16,
                               kind="ExternalOutput")
        rdbg = nc.dram_tensor("rdbg", [128, CH], F32R, kind="ExternalOutput")
        bdbg = nc.dram_tensor("bdbg", [128, CH], F32, kind="ExternalOutput")
        adbg = nc.dram_tensor("adbg", [128, CH], BF16, kind="ExternalOutput")

    swap = [i ^ 1 for i in range(32)]

    with nc.allow_low_precision(reason="bf16 matmuls are intended"), \
         tile.TileContext(nc) as tc:
        with (
            tc.tile_pool(name="consts", bufs=1) as cp,
            tc.tile_pool(name="qt", bufs=1) as qtp,
            tc.tile_pool(name="at", bufs=1) as atp,
            tc.tile_pool(name="psbig", bufs=2, space="PSUM") as psb,
            tc.tile_pool(name="psav", bufs=1, space="PSUM") as psa,
        ):
            cos_sb = cp.tile([128, CH], BF16)
            nc.sync.dma_start(cos_sb[:], cosT[:])
            sin_sb = cp.tile([128, CH], BF16)
            nc.sync.dma_start(sin_sb[:], sinT[:])
            qts = []
            ats = []

            # ---- Phases A+B interleaved per head pair: q-projection +
            # RoPE for pair m, immediately followed by its attention.
            # Keeps the PE matmul stream dense (HAM stays warm) and
            # overlaps the exp chain with the next pair's projection.
            with (
                tc.tile_pool(name="xts", bufs=1) as xp,
                tc.tile_pool(name="wq", bufs=2) as wp,
                tc.tile_pool(name="rope", bufs=1) as rp,
                tc.tile_pool(name="kv", bufs=1) as kp,
                tc.tile_pool(name="expp", bufs=4) as ep,
                tc.tile_pool(name="rcp", bufs=2) as rcp,
            ):
                xts = xp.tile([128, DT, CH], BF16)
                nc.sync.dma_start(xts[:], xr[:])
                wq_cs = []
                for wc in range(c["WC"]):
                    wq_c = wp.tile([128, DT, 512], BF16, tag="wq")
                    nc.sync.dma_start(wq_c[:], wqr[:, wc])
                    wq_cs.append(wq_c)
                ktd_sb = kp.tile([128, KV, KB, 128], BF16)
                nc.sync.dma_start(ktd_sb[0:64], kT[:])
                nc.sync.dma_start(ktd_sb[64:128], kT[:])
                va0_sb = kp.tile([128, KV, KB, 128], BF16)
                nc.sync.dma_start(va0_sb[:], vaug0[:])
                va1_sb = kp.tile([128, KV, KB, 128], BF16)
                nc.sync.dma_start(va1_sb[:], vaug1[:])
                tri_sb = kp.tile([128, 128], BF16)
                nc.sync.dma_start(tri_sb[:], tri[:])
                br_sb = kp.tile([128, CH], BF16)
                nc.sync.dma_start(br_sb[0:KB], brow[:])
                nc.sync.dma_start(br_sb[64:64 + KB], brow[:])
                sel_sb = kp.tile([128, KB, 128], BF16)
                nc.sync.dma_start(sel_sb[0:KB], sel[:])
                nc.sync.dma_start(sel_sb[64:64 + KB], sel[:])
                ebc_sb = kp.tile([2, 128], F32R)
                nc.sync.dma_start(ebc_sb[:], ebc[:])

                for m in range(NP):
                    wq_c = wq_cs[m // 4]
                    me = 128 * (m % 4)
                    qp = psb.tile([128, CH], F32, tag="big")
                    for kt in range(DT):
                        for n0, nn in c["NC"]:
                            nc.tensor.matmul(
                                qp[:, n0:n0 + nn],
                                wq_c[:, kt, me:me + 128],
                                xts[:, kt, n0:n0 + nn],
                                start=(kt == 0), stop=(kt == DT - 1))
                    qcp = rp.tile([128, CH], F32, tag="qcp")
                    nc.scalar.copy(qcp[:], qp[:])
                    qs = rp.tile([128, CH], F32, tag="qs")
                    nc.vector.stream_shuffle(qs[:], qcp[:], swap)
                    t1 = rp.tile([128, CH], F32, tag="t1")
                    nc.vector.tensor_mul(t1[:], qcp[:], cos_sb[:])
                    t2 = rp.tile([128, CH], F32, tag="t2")
                    nc.gpsimd.tensor_mul(t2[:], qs[:], sin_sb[:])
                    qt = qtp.tile([128, CH], BF16, tag="qt", bufs=2)
                    nc.gpsimd.tensor_add(qt[:], t1[:], t2[:])
                    if dbg and m == 0:
                        nc.sync.dma_start(qdbg[:], qt[:])
                    qts.append(qt)

                    kv = (2 * m) // hpkv
                    av0 = psa.tile([128, CH], F32, tag="av0")
                    av1 = psa.tile([128, CH], F32, tag="av1")
                    for kb in range(KB):
                        # both heads' QK back-to-back: disjoint PE row
                        # groups (0:64 / 64:128) execute concurrently
                        sps = []
                        for hh in range(2):
                            sp = psb.tile([128, CH], F32, tag="big")
                            lh = ktd_sb[64 * hh:64 * (hh + 1), kv, kb, :]
                            rh = qts[m][64 * hh:64 * (hh + 1)]
                            for n0, nn in c["NC"]:
                                mb = min(128 * kb, n0 + nn)
                                nc.tensor.matmul(
                                    sp[:, n0:n0 + nn], lh,
                                    rh[:, n0:n0 + nn],
                                    start=True, stop=not (mb > n0))
                            sps.append(sp)
                        # causal column mask: bias-row matmuls on
                        # row groups 0:8 / 64:72 (concurrent)
                        for hh in range(2):
                            for n0, nn in c["NC"]:
                                mb = min(128 * kb, n0 + nn)
                                if mb > n0:
                                    nc.tensor.matmul(
                                        sps[hh][:, n0:mb],
                                        sel_sb[64 * hh:64 * hh + KB, kb, :],
                                        br_sb[64 * hh:64 * hh + KB, n0:mb],
                                        start=False, stop=True)
                        ers = []
                        for hh in range(2):
                            er = ep.tile([128, CH], BF16, tag="er")
                            nc.scalar.activation(er[:], sps[hh][:], AF.Exp)
                            if 128 * (kb + 1) <= CH:
                                dsl = slice(128 * kb, 128 * (kb + 1))
                                nc.vector.tensor_mul(er[:, dsl], er[:, dsl],
                                                     tri_sb[:])
                            if dbg and m == 0 and kb == 0:
                                nc.sync.dma_start(
                                    (edbg if hh == 0 else e1dbg)[:], er[:])
                            ers.append(er)
                        for n0, nn in c["NC"]:
                            nc.tensor.matmul(
                                av0[:, n0:n0 + nn],
                                va0_sb[:, kv, kb, :], ers[0][:, n0:n0 + nn],
                                start=(kb == 0), stop=(kb == KB - 1))
                        for n0, nn in c["NC"]:
                            nc.tensor.matmul(
                                av1[:, n0:n0 + nn],
                                va1_sb[:, kv, kb, :], ers[1][:, n0:n0 + nn],
                                start=(kb == 0), stop=(kb == KB - 1))
                    # normalize: den_h0 at av0 row 64, den_h1 at av1 row 0.
                    # Stage V-rows + den rows out of PSUM, one 2-lane
                    # reciprocal, broadcast via a tiny K=2 matmul.
                    au = rcp.tile([128, CH], F32R, tag="au")
                    nc.vector.tensor_copy(au[0:64], av0[0:64])
                    nc.vector.tensor_copy(au[64:128], av1[64:128])
                    stg = rcp.tile([128, CH], BF16, tag="stg", bufs=1)
                    nc.scalar.copy(stg[64:65], av0[64:65])
                    nc.scalar.copy(stg[0:1], av1[0:1])
                    den = rcp.tile([2, CH], BF16, tag="den", bufs=1)
                    nc.sync.dma_start(den[0:1], stg[64:65])
                    nc.sync.dma_start(den[1:2], stg[0:1])
                    rec = rcp.tile([2, CH], F32R, tag="rec")
                    nc.vector.reciprocal(rec[:], den[:])
                    bc = psb.tile([128, CH], F32, tag="big")
                    for n0, nn in c["NC"]:
                        nc.tensor.matmul(bc[:, n0:n0 + nn], ebc_sb[:],
                                         rec[:, n0:n0 + nn],
                                         start=True, stop=True)
                    at = atp.tile([128, CH], BF16, tag=f"at{m}")
                    nc.vector.tensor_mul(at[:], au[:], bc[:])
                    if dbg and m == 0:
                        nc.sync.dma_start(rdbg[0:2], rec[:])
                        nc.sync.dma_start(adbg[:], at[:])
                    ats.append(at)

            # ---- Phase C: out[q, o] = sum_m at_m.T @ woT_m
            with (
                tc.tile_pool(name="wo", bufs=2) as wop,
                tc.tile_pool(name="osb", bufs=3) as op_,
            ):
                MQ = CH // 128
                for oc, (o0, ow) in enumerate(c["OC"]):
                    wo_c = wop.tile([128, NP, 512], BF16, tag="wo")
                    nc.sync.dma_start(wo_c[:], wor[:, oc])
                    for mq in range(MQ):
                        opx = psb.tile([128, 512], F32, tag="big")
                        for kq in range(NP):
                            nc.tensor.matmul(
                                opx[:, :ow],
                                ats[kq][:, 128 * mq:128 * (mq + 1)],
                                wo_c[:, kq, :ow],
                                start=(kq == 0), stop=(kq == NP - 1))
                        osb = op_.tile([128, 512], F32, tag="os")
                        if mq % 2 == 0:
                            nc.scalar.copy(osb[:, :ow], opx[:, :ow])
                        else:
                            nc.vector.tensor_copy(osb[:, :ow], opx[:, :ow])
                        nc.sync.dma_start(
                            out[128 * mq:128 * (mq + 1), o0:o0 + ow],
                            osb[:, :ow])
    nc.compile()
    return nc


def host_inputs(cfg, x, k_cache, v_cache, Wq, Wo, core):
    import ml_dtypes
    c = _derived(cfg)
    CH, KB, KV, W, DH = c["CH"], c["KB"], c["KV"], c["W"], c["DH"]
    b, ch = core // 4, core % 4
    Tc = k_cache.shape[2]
    f32 = np.float32
    bf16 = ml_dtypes.bfloat16

    DT, NP, D = c["DT"], c["NP"], c["D"]
    xT = x[b, CH * ch:CH * (ch + 1), :].T            # (D, CH)
    xr = np.ascontiguousarray(
        xT.reshape(DT, 128, CH).transpose(1, 0, 2)).astype(bf16)
    WC = c["WC"]
    wqT = Wq.T * f32(1.0 / np.sqrt(DH))              # (D, D)
    wqr = np.ascontiguousarray(
        wqT.reshape(DT, 128, WC, 512).transpose(1, 2, 0, 3)).astype(bf16)
    woT = Wo.T                                       # (D, D)
    wor = np.ascontiguousarray(
        woT.reshape(NP, 128, WC, 512).transpose(1, 2, 0, 3)).astype(bf16)
    kw = k_cache[b, :, Tc - W:, :]                      # (KV, W, DH)
    kT = np.ascontiguousarray(
        kw.reshape(KV, KB, 128, DH).transpose(3, 0, 1, 2)).astype(bf16)
    vw = v_cache[b, :, Tc - W:, :].reshape(KV, KB, 128, DH)
    vaug0 = np.zeros((128, KV, KB, 128), f32)
    vaug0[:, :, :, :DH] = vw.transpose(2, 0, 1, 3)
    vaug0[:, :, :, DH] = 1.0
    vaug1 = np.zeros((128, KV, KB, 128), f32)
    vaug1[:, :, :, 64:128] = vw.transpose(2, 0, 1, 3)
    vaug1[:, :, :, 0] = 1.0
    pos = (CH * ch + np.arange(CH)).astype(f32)
    inv = 1.0 / (cfg["BASE"] ** (np.arange(0, DH, 2, dtype=f32) / DH))
    r = np.arange(128)
    u = (r % 64) // 2
    ang = pos[None, :] * inv[u][:, None]                # (128, CH)
    cosT = np.cos(ang).astype(bf16)
    sinT = (np.sin(ang) * np.where(r % 2 == 0, -1.0, 1.0)[:, None]).astype(bf16)
    if ch == 0:
        tri = (np.arange(128)[:, None] <= np.arange(128)[None, :]).astype(f32)
        brow = np.zeros((KB, CH), f32)
        for kb in range(KB):
            brow[kb, :128 * kb] = BIGNEG
    else:
        tri = np.ones((128, 128), f32)
        brow = np.zeros((KB, CH), f32)
    sel = np.zeros((KB, KB, 128), f32)
    for kb in range(KB):
        sel[kb, kb, :] = 1.0
    ebc = np.zeros((2, 128), f32)
    ebc[0, 0:64] = 1.0
    ebc[1, 64:128] = 1.0
    return {"xr": xr, "wqr": wqr, "wor": wor, "kT": kT,
            "vaug0": vaug0.astype(bf16), "vaug1": vaug1.astype(bf16),
            "cosT": cosT, "sinT": sinT, "tri": tri.astype(bf16),
            "brow": brow.astype(bf16), "sel": sel.astype(bf16), "ebc": ebc}


_NC_CACHE = {}


def run(cfg, x, k_cache, v_cache, Wq, Wo, trace=False):
    key = tuple(sorted((k, v) for k, v in cfg.items()))
    if key not in _NC_CACHE:
        _NC_CACHE[key] = build(cfg)
    nc = _NC_CACHE[key]
    in_maps = [host_inputs(cfg, x, k_cache, v_cache, Wq, Wo, c)
               for c in range(8)]
    res = None
    for attempt in range(3):
        try:
            res = run_bass_kernel_spmd(nc, in_maps, core_ids=list(range(8)),
                                       trace=trace)
            break
        except Exception:
            if attempt == 2:
                raise
            time.sleep(2.0)
    outs = [res.results[c]["out"] for c in range(8)]
    full = np.stack([np.concatenate(outs[0:4], axis=0),
                     np.concatenate(outs[4:8], axis=0)])
    return full, res


def kernel(x, k_cache, v_cache, Wq, Wo):
    full, _ = run(FULL, np.asarray(x), np.asarray(k_cache),
                  np.asarray(v_cache), np.asarray(Wq), np.asarray(Wo))
    return full.astype(np.float32)


# revision 54
# speedup vs baseline: 1.0860x; 1.0451x over previous
"""Self-contained Trainium2 Bass kernel for nn_AttnLayer_71382356460296.

Sharding: data-parallel over batch B (2) x sequence-parallel over query
chunks (4) => 8 cores. Each core computes its (batch, 1024-query chunk)
slice of the full layer: q-projection + RoPE, windowed GQA attention with
a top-left-aligned causal mask, and the output projection. No reduction
is needed across cores - the host just concatenates the 8 output chunks.

v2: bf16 matmul operands (fp32 PSUM accumulation), attention kept fully
in SBUF (no DRAM round-trip), causal column-mask applied via
row-concurrent bias matmuls (partitions 0:8 / 64:72), softmax
normalization via per-head reciprocal + gpsimd partition_broadcast, and
the output projection streams Wo against attention tiles used as the
stationary operand.
"""

import time

import numpy as np

import concourse.bacc as bacc
import concourse.mybir as mybir
import concourse.tile as tile
from concourse.bass_utils import run_bass_kernel_spmd

F32 = mybir.dt.float32
F32R = mybir.dt.float32r
BF16 = mybir.dt.bfloat16
AF = mybir.ActivationFunctionType

FULL = dict(B=2, T=4096, D=2048, H=32, KV=8, DH=64, W=1024, BASE=10000.0)
BIGNEG = -1e30


def _derived(cfg):
    d = dict(cfg)
    d["CH"] = cfg["T"] // 4            # queries per core
    d["KB"] = cfg["W"] // 128          # 128-key blocks in window
    d["DT"] = cfg["D"] // 128          # contraction tiles for Wq
    d["NP"] = cfg["H"] // 2            # head pairs (= D/128 output tiles)
    d["NC"] = [(i, min(512, d["CH"] - i)) for i in range(0, d["CH"], 512)]
    d["OC"] = [(i, min(512, cfg["D"] - i)) for i in range(0, cfg["D"], 512)]
    d["WC"] = cfg["D"] // 512
    assert d["NP"] * 128 == cfg["D"]
    return d


def build(cfg, dbg=False):
    c = _derived(cfg)
    CH, KB, DT, NP, KV, H = c["CH"], c["KB"], c["DT"], c["NP"], c["KV"], c["H"]
    hpkv = H // KV
    nc = bacc.Bacc("TRN2", target_bir_lowering=False, debug=False)

    # host-rearranged: xr[p, kt, f] = xT[kt*128+p, f]; wqr[p, kt, e] =
    # wqT[kt*128+p, e] (chunked on e); wor[p, oc, m, o] = woT[m*128+p,
    # oc*512+o] — every DMA is one contiguous run per partition.
    xr = nc.dram_tensor("xr", [128, DT, CH], BF16, kind="ExternalInput")
    wqr = nc.dram_tensor("wqr", [128, c["WC"], DT, 512], BF16,
                         kind="ExternalInput")
    wor = nc.dram_tensor("wor", [128, c["WC"], NP, 512], BF16,
                         kind="ExternalInput")
    kT = nc.dram_tensor("kT", [64, KV, KB, 128], BF16, kind="ExternalInput")
    vaug0 = nc.dram_tensor("vaug0", [128, KV, KB, 128], BF16,
                           kind="ExternalInput")
    vaug1 = nc.dram_tensor("vaug1", [128, KV, KB, 128], BF16,
                           kind="ExternalInput")
    cosT = nc.dram_tensor("cosT", [128, CH], BF16, kind="ExternalInput")
    sinT = nc.dram_tensor("sinT", [128, CH], BF16, kind="ExternalInput")
    tri = nc.dram_tensor("tri", [128, 128], BF16, kind="ExternalInput")
    brow = nc.dram_tensor("brow", [KB, CH], BF16, kind="ExternalInput")
    sel = nc.dram_tensor("sel", [KB, KB, 128], BF16, kind="ExternalInput")
    ebc = nc.dram_tensor("ebc", [2, 128], F32R, kind="ExternalInput")
    out = nc.dram_tensor("out", [CH, c["D"]], F32, kind="ExternalOutput")
    if dbg:
        qdbg = nc.dram_tensor("qdbg", [128, CH], BF16, kind="ExternalOutput")
        edbg = nc.dram_tensor("edbg", [128, CH], BF16, kind="ExternalOutput")
        e1dbg = nc.dram_tensor("e1dbg", [128, CH], BF16,
                               kind="ExternalOutput")
        rdbg = nc.dram_tensor("rdbg", [128, CH], F32R, kind="ExternalOutput")
        bdbg = nc.dram_tensor("bdbg", [128, CH], F32, kind="ExternalOutput")
        adbg = nc.dram_tensor("adbg", [128, CH], BF16, kind="ExternalOutput")

    swap = [i ^ 1 for i in range(32)]

    with nc.allow_low_precision(reason="bf16 matmuls are intended"), \
         tile.TileContext(nc) as tc:
        with (
            tc.tile_pool(name="consts", bufs=1) as cp,
            tc.tile_pool(name="qt", bufs=1) as qtp,
            tc.tile_pool(name="at", bufs=1) as atp,
            tc.tile_pool(name="psbig", bufs=2, space="PSUM") as psb,
            tc.tile_pool(name="psav", bufs=1, space="PSUM") as psa,
        ):
            cos_sb = cp.tile([128, CH], BF16)
            nc.sync.dma_start(cos_sb[:], cosT[:])
            sin_sb = cp.tile([128, CH], BF16)
            nc.sync.dma_start(sin_sb[:], sinT[:])
            qts = []
            ats = []

            # ---- Phases A+B interleaved per head pair: q-projection +
            # RoPE for pair m, immediately followed by its attention.
            # Keeps the PE matmul stream dense (HAM stays warm) and
            # overlaps the exp chain with the next pair's projection.
            with (
                tc.tile_pool(name="xts", bufs=1) as xp,
                tc.tile_pool(name="wq", bufs=2) as wp,
                tc.tile_pool(name="rope", bufs=1) as rp,
                tc.tile_pool(name="kv", bufs=1) as kp,
                tc.tile_pool(name="expp", bufs=4) as ep,
                tc.tile_pool(name="rcp", bufs=2) as rcp,
            ):
                xts = xp.tile([128, DT, CH], BF16)
                nc.sync.dma_start(xts[:], xr[:])
                wq_cs = []
                for wc in range(c["WC"]):
                    wq_c = wp.tile([128, DT, 512], BF16, tag="wq")
                    nc.sync.dma_start(wq_c[:], wqr[:, wc])
                    wq_cs.append(wq_c)
                ktd_sb = kp.tile([128, KV, KB, 128], BF16)
                nc.sync.dma_start(ktd_sb[0:64], kT[:])
                nc.sync.dma_start(ktd_sb[64:128], kT[:])
                va0_sb = kp.tile([128, KV, KB, 128], BF16)
                nc.sync.dma_start(va0_sb[:], vaug0[:])
                va1_sb = kp.tile([128, KV, KB, 128], BF16)
                nc.sync.dma_start(va1_sb[:], vaug1[:])
                tri_sb = kp.tile([128, 128], BF16)
                nc.sync.dma_start(tri_sb[:], tri[:])
                br_sb = kp.tile([128, CH], BF16)
                nc.sync.dma_start(br_sb[0:KB], brow[:])
                nc.sync.dma_start(br_sb[64:64 + KB], brow[:])
                sel_sb = kp.tile([128, KB, 128], BF16)
                nc.sync.dma_start(sel_sb[0:KB], sel[:])
                nc.sync.dma_start(sel_sb[64:64 + KB], sel[:])
                ebc_sb = kp.tile([2, 128], F32R)
                nc.sync.dma_start(ebc_sb[:], ebc[:])

                for m in range(NP):
                    wq_c = wq_cs[m // 4]
                    me = 128 * (m % 4)
                    qp = psb.tile([128, CH], F32, tag="big")
                    for kt in range(DT):
                        for n0, nn in c["NC"]:
                            nc.tensor.matmul(
                                qp[:, n0:n0 + nn],
                                wq_c[:, kt, me:me + 128],
                                xts[:, kt, n0:n0 + nn],
                                start=(kt == 0), stop=(kt == DT - 1))
                    qcp = rp.tile([128, CH], F32, tag="qcp")
                    nc.scalar.copy(qcp[:], qp[:])
                    qs = rp.tile([128, CH], F32, tag="qs")
                    nc.vector.stream_shuffle(qs[:], qcp[:], swap)
                    t1 = rp.tile([128, CH], F32, tag="t1")
                    nc.vector.tensor_mul(t1[:], qcp[:], cos_sb[:])
                    t2 = rp.tile([128, CH], F32, tag="t2")
                    nc.gpsimd.tensor_mul(t2[:], qs[:], sin_sb[:])
                    qt = qtp.tile([128, CH], BF16, tag="qt", bufs=2)
                    nc.gpsimd.tensor_add(qt[:], t1[:], t2[:])
                    if dbg and m == 0:
                        nc.sync.dma_start(qdbg[:], qt[:])
                    qts.append(qt)

                    kv = (2 * m) // hpkv
                    av0 = psa.tile([128, CH], F32, tag="av0")
                    av1 = psa.tile([128, CH], F32, tag="av1")
                    for kb in range(KB):
                        # both heads' QK back-to-back: disjoint PE row
                        # groups (0:64 / 64:128) execute concurrently
                        sps = []
                        for hh in range(2):
                            sp = psb.tile([128, CH], F32, tag="big")
                            lh = ktd_sb[64 * hh:64 * (hh + 1), kv, kb, :]
                            rh = qts[m][64 * hh:64 * (hh + 1)]
                            for n0, nn in c["NC"]:
                                mb = min(128 * kb, n0 + nn)
                                nc.tensor.matmul(
                                    sp[:, n0:n0 + nn], lh,
                                    rh[:, n0:n0 + nn],
                                    start=True, stop=not (mb > n0))
                            sps.append(sp)
                        # causal column mask: bias-row matmuls on
                        # row groups 0:8 / 64:72 (concurrent)
                        for hh in range(2):
                            for n0, nn in c["NC"]:
                                mb = min(128 * kb, n0 + nn)
                                if mb > n0:
                                    nc.tensor.matmul(
                                        sps[hh][:, n0:mb],
                                        sel_sb[64 * hh:64 * hh + KB, kb, :],
                                        br_sb[64 * hh:64 * hh + KB, n0:mb],
                                        start=False, stop=True)
                        ers = []
                        for hh in range(2):
                            er = ep.tile([128, CH], BF16, tag="er")
                            nc.scalar.activation(er[:], sps[hh][:], AF.Exp)
                            if 128 * (kb + 1) <= CH:
                                dsl = slice(128 * kb, 128 * (kb + 1))
                                nc.vector.tensor_mul(er[:, dsl], er[:, dsl],
                                                     tri_sb[:])
                            if dbg and m == 0 and kb == 0:
                                nc.sync.dma_start(
                                    (edbg if hh == 0 else e1dbg)[:], er[:])
                            ers.append(er)
                        for n0, nn in c["NC"]:
                            nc.tensor.matmul(
                                av0[:, n0:n0 + nn],
                                va0_sb[:, kv, kb, :], ers[0][:, n0:n0 + nn],
                                start=(kb == 0), stop=(kb == KB - 1))
                        for n0, nn in c["NC"]:
                            nc.tensor.matmul(
                                av1[:, n0:n0 + nn],
                                va1_sb[:, kv, kb, :], ers[1][:, n0:n0 + nn],
                                start=(kb == 0), stop=(kb == KB - 1))
                    # normalize: den_h0 at av0 row 64, den_h1 at av1 row 0.
                    # Stage V-rows + dens out of PSUM, broadcast raw dens
                    # via two K=1 matmuls into the freed av0 ring slot,
                    # then ONE fast-approx reciprocal over the full tile
                    # (vector.reciprocal costs 6.5us/call on HW; the
                    # custom fast op is only safe at base partition 0).
                    au = rcp.tile([128, CH], F32R, tag="au")
                    nc.vector.tensor_copy(au[0:64], av0[0:64])
                    nc.vector.tensor_copy(au[64:128], av1[64:128])
                    stg = rcp.tile([128, CH], F32R, tag="stg", bufs=1)
                    nc.scalar.copy(stg[64:65], av0[64:65])
                    nc.scalar.copy(stg[0:1], av1[0:1])
                    den2 = rcp.tile([2, CH], F32R, tag="den2", bufs=1)
                    nc.sync.dma_start(den2[0:1], stg[64:65])
                    nc.sync.dma_start(den2[1:2], stg[0:1])
                    bcd = psa.tile([128, CH], F32, tag="av0")
                    for n0, nn in c["NC"]:
                        nc.tensor.matmul(bcd[:, n0:n0 + nn], ebc_sb[:],
                                         den2[:, n0:n0 + nn],
                                         start=True, stop=True)
                    bcs = rcp.tile([128, CH], F32, tag="bcs", bufs=1)
                    nc.vector.reciprocal_approx_fast(bcs[:], bcd[:])
                    at = atp.tile([128, CH], BF16, tag=f"at{m}")
                    nc.vector.tensor_mul(at[:], au[:], bcs[:])
                    if dbg and m == 0:
                        nc.sync.dma_start(rdbg[0:1], bcs[0:1])
                        nc.sync.dma_start(rdbg[1:2], bcs[127:128])
                        nc.sync.dma_start(adbg[:], at[:])
                    ats.append(at)

            # ---- Phase C: out[q, o] = sum_m at_m.T @ woT_m
            with (
                tc.tile_pool(name="wo", bufs=2) as wop,
                tc.tile_pool(name="osb", bufs=3) as op_,
            ):
                MQ = CH // 128
                for oc, (o0, ow) in enumerate(c["OC"]):
                    wo_c = wop.tile([128, NP, 512], BF16, tag="wo")
                    nc.sync.dma_start(wo_c[:], wor[:, oc])
                    for mq in range(MQ):
                        opx = psb.tile([128, 512], F32, tag="big")
                        for kq in range(NP):
                            nc.tensor.matmul(
                                opx[:, :ow],
                                ats[kq][:, 128 * mq:128 * (mq + 1)],
                                wo_c[:, kq, :ow],
                                start=(kq == 0), stop=(kq == NP - 1))
                        osb = op_.tile([128, 512], F32, tag="os")
                        if mq % 2 == 0:
                            nc.scalar.copy(osb[:, :ow], opx[:, :ow])
                        else:
                            nc.vector.tensor_copy(osb[:, :ow], opx[:, :ow])
                        nc.sync.dma_start(
                            out[128 * mq:128 * (mq + 1), o0:o0 + ow],
                            osb[:, :ow])
    nc.compile()
    return nc


def host_inputs(cfg, x, k_cache, v_cache, Wq, Wo, core):
    import ml_dtypes
    c = _derived(cfg)
    CH, KB, KV, W, DH = c["CH"], c["KB"], c["KV"], c["W"], c["DH"]
    b, ch = core // 4, core % 4
    Tc = k_cache.shape[2]
    f32 = np.float32
    bf16 = ml_dtypes.bfloat16

    DT, NP, D = c["DT"], c["NP"], c["D"]
    xT = x[b, CH * ch:CH * (ch + 1), :].T            # (D, CH)
    xr = np.ascontiguousarray(
        xT.reshape(DT, 128, CH).transpose(1, 0, 2)).astype(bf16)
    WC = c["WC"]
    wqT = Wq.T * f32(1.0 / np.sqrt(DH))              # (D, D)
    wqr = np.ascontiguousarray(
        wqT.reshape(DT, 128, WC, 512).transpose(1, 2, 0, 3)).astype(bf16)
    woT = Wo.T                                       # (D, D)
    wor = np.ascontiguousarray(
        woT.reshape(NP, 128, WC, 512).transpose(1, 2, 0, 3)).astype(bf16)
    kw = k_cache[b, :, Tc - W:, :]                      # (KV, W, DH)
    kT = np.ascontiguousarray(
        kw.reshape(KV, KB, 128, DH).transpose(3, 0, 1, 2)).astype(bf16)
    vw = v_cache[b, :, Tc - W:, :].reshape(KV, KB, 128, DH)
    vaug0 = np.zeros((128, KV, KB, 128), f32)
    vaug0[:, :, :, :DH] = vw.transpose(2, 0, 1, 3)
    vaug0[:, :, :, DH] = 1.0
    vaug1 = np.zeros((128, KV, KB, 128), f32)
    vaug1[:, :, :, 64:128] = vw.transpose(2, 0, 1, 3)
    vaug1[:, :, :, 0] = 1.0
    pos = (CH * ch + np.arange(CH)).astype(f32)
    inv = 1.0 / (cfg["BASE"] ** (np.arange(0, DH, 2, dtype=f32) / DH))
    r = np.arange(128)
    u = (r % 64) // 2
    ang = pos[None, :] * inv[u][:, None]                # (128, CH)
    cosT = np.cos(ang).astype(bf16)
    sinT = (np.sin(ang) * np.where(r % 2 == 0, -1.0, 1.0)[:, None]).astype(bf16)
    if ch == 0:
        tri = (np.arange(128)[:, None] <= np.arange(128)[None, :]).astype(f32)
        brow = np.zeros((KB, CH), f32)
        for kb in range(KB):
            brow[kb, :128 * kb] = BIGNEG
    else:
        tri = np.ones((128, 128), f32)
        brow = np.zeros((KB, CH), f32)
    sel = np.zeros((KB, KB, 128), f32)
    for kb in range(KB):
        sel[kb, kb, :] = 1.0
    ebc = np.zeros((2, 128), f32)
    ebc[0, 0:64] = 1.0
    ebc[1, 64:128] = 1.0
    return {"xr": xr, "wqr": wqr, "wor": wor, "kT": kT,
            "vaug0": vaug0.astype(bf16), "vaug1": vaug1.astype(bf16),
            "cosT": cosT, "sinT": sinT, "tri": tri.astype(bf16),
            "brow": brow.astype(bf16), "sel": sel.astype(bf16), "ebc": ebc}


_NC_CACHE = {}


def run(cfg, x, k_cache, v_cache, Wq, Wo, trace=False):
    key = tuple(sorted((k, v) for k, v in cfg.items()))
    if key not in _NC_CACHE:
        _NC_CACHE[key] = build(cfg)
    nc = _NC_CACHE[key]
    in_maps = [host_inputs(cfg, x, k_cache, v_cache, Wq, Wo, c)
               for c in range(8)]
    res = None
    for attempt in range(3):
        try:
            res = run_bass_kernel_spmd(nc, in_maps, core_ids=list(range(8)),
                                       trace=trace)
            break
        except Exception:
            if attempt == 2:
                raise
            time.sleep(2.0)
    outs = [res.results[c]["out"] for c in range(8)]
    full = np.stack([np.concatenate(outs[0:4], axis=0),
                     np.concatenate(outs[4:8], axis=0)])
    return full, res


def kernel(x, k_cache, v_cache, Wq, Wo):
    full, _ = run(FULL, np.asarray(x), np.asarray(k_cache),
                  np.asarray(v_cache), np.asarray(Wq), np.asarray(Wo))
    return full.astype(np.float32)


# revision 63
# speedup vs baseline: 1.4891x; 1.3711x over previous
"""Self-contained Trainium2 Bass kernel for nn_AttnLayer_71382356460296.

Sharding: data-parallel over batch B (2) x sequence-parallel over query
chunks (4) => 8 cores. Each core computes its (batch, 1024-query chunk)
slice of the full layer: q-projection + RoPE, windowed GQA attention with
a top-left-aligned causal mask, and the output projection. No reduction
is needed across cores - the host just concatenates the 8 output chunks.

v2: bf16 matmul operands (fp32 PSUM accumulation), attention kept fully
in SBUF (no DRAM round-trip), causal column-mask applied via
row-concurrent bias matmuls (partitions 0:8 / 64:72), softmax
normalization via per-head reciprocal + gpsimd partition_broadcast, and
the output projection streams Wo against attention tiles used as the
stationary operand.
"""

import time

import numpy as np

import concourse.bacc as bacc
import concourse.mybir as mybir
import concourse.tile as tile
from concourse.bass_utils import run_bass_kernel_spmd

F32 = mybir.dt.float32
F32R = mybir.dt.float32r
BF16 = mybir.dt.bfloat16
AF = mybir.ActivationFunctionType

FULL = dict(B=2, T=4096, D=2048, H=32, KV=8, DH=64, W=1024, BASE=10000.0)
BIGNEG = -1e30


def _derived(cfg):
    d = dict(cfg)
    d["CH"] = cfg["T"] // 4            # queries per core
    d["KB"] = cfg["W"] // 128          # 128-key blocks in window
    d["DT"] = cfg["D"] // 128          # contraction tiles for Wq
    d["NP"] = cfg["H"] // 2            # head pairs (= D/128 output tiles)
    d["NC"] = [(i, min(512, d["CH"] - i)) for i in range(0, d["CH"], 512)]
    d["OC"] = [(i, min(512, cfg["D"] - i)) for i in range(0, cfg["D"], 512)]
    d["WC"] = cfg["D"] // 512
    assert d["NP"] * 128 == cfg["D"]
    return d


def build(cfg, dbg=False):
    c = _derived(cfg)
    CH, KB, DT, NP, KV, H = c["CH"], c["KB"], c["DT"], c["NP"], c["KV"], c["H"]
    hpkv = H // KV
    nc = bacc.Bacc("TRN2", target_bir_lowering=False, debug=False)

    # host-rearranged: xr[p, kt, f] = xT[kt*128+p, f]; wqr[p, kt, e] =
    # wqT[kt*128+p, e] (chunked on e); wor[p, oc, m, o] = woT[m*128+p,
    # oc*512+o] — every DMA is one contiguous run per partition.
    xr = nc.dram_tensor("xr", [128, DT, CH], BF16, kind="ExternalInput")
    wqr = nc.dram_tensor("wqr", [128, c["WC"], DT, 512], BF16,
                         kind="ExternalInput")
    wor = nc.dram_tensor("wor", [128, c["WC"], NP, 512], BF16,
                         kind="ExternalInput")
    # rows 0:64 = K head-dims; rows 64:72 = per-kb bias selector ones
    kT = nc.dram_tensor("kT", [64 + KB, KV, KB, 128], BF16,
                        kind="ExternalInput")
    vaug0 = nc.dram_tensor("vaug0", [128, KV, KB, 128], BF16,
                           kind="ExternalInput")
    vaug1 = nc.dram_tensor("vaug1", [128, KV, KB, 128], BF16,
                           kind="ExternalInput")
    cosT = nc.dram_tensor("cosT", [128, CH], BF16, kind="ExternalInput")
    sinT = nc.dram_tensor("sinT", [128, CH], BF16, kind="ExternalInput")
    tri = nc.dram_tensor("tri", [128, 128], BF16, kind="ExternalInput")
    brow = nc.dram_tensor("brow", [KB, CH], BF16, kind="ExternalInput")
    ebc = nc.dram_tensor("ebc", [2, 128], F32R, kind="ExternalInput")
    out = nc.dram_tensor("out", [CH, c["D"]], F32, kind="ExternalOutput")
    if dbg:
        qdbg = nc.dram_tensor("qdbg", [128, CH], BF16, kind="ExternalOutput")
        edbg = nc.dram_tensor("edbg", [128, CH], BF16, kind="ExternalOutput")
        e1dbg = nc.dram_tensor("e1dbg", [128, CH], BF16,
                               kind="ExternalOutput")
        rdbg = nc.dram_tensor("rdbg", [128, CH], F32R, kind="ExternalOutput")
        bdbg = nc.dram_tensor("bdbg", [128, CH], F32, kind="ExternalOutput")
        adbg = nc.dram_tensor("adbg", [128, CH], BF16, kind="ExternalOutput")

    swap = [i ^ 1 for i in range(32)]

    with nc.allow_low_precision(reason="bf16 matmuls are intended"), \
         tile.TileContext(nc) as tc:
        with (
            tc.tile_pool(name="consts", bufs=1) as cp,
            tc.tile_pool(name="qt", bufs=1) as qtp,
            tc.tile_pool(name="at", bufs=1) as atp,
            tc.tile_pool(name="psbig", bufs=2, space="PSUM") as psb,
            tc.tile_pool(name="psav", bufs=1, space="PSUM") as psa,
        ):
            cos_sb = cp.tile([128, CH], BF16)
            nc.sync.dma_start(cos_sb[:], cosT[:])
            sin_sb = cp.tile([128, CH], BF16)
            nc.sync.dma_start(sin_sb[:], sinT[:])
            qts = []
            ats = []

            # ---- Phases A+B interleaved per head pair: q-projection +
            # RoPE for pair m, immediately followed by its attention.
            # Keeps the PE matmul stream dense (HAM stays warm) and
            # overlaps the exp chain with the next pair's projection.
            with (
                tc.tile_pool(name="xts", bufs=1) as xp,
                tc.tile_pool(name="wq", bufs=2) as wp,
                tc.tile_pool(name="rope", bufs=1) as rp,
                tc.tile_pool(name="kv", bufs=1) as kp,
                tc.tile_pool(name="expp", bufs=4) as ep,
                tc.tile_pool(name="rcp", bufs=2) as rcp,
            ):
                xts = xp.tile([128, DT, CH], BF16)
                nc.sync.dma_start(xts[:], xr[:])
                wq_cs = []
                for wc in range(c["WC"]):
                    wq_c = wp.tile([128, DT, 512], BF16, tag="wq")
                    nc.sync.dma_start(wq_c[:], wqr[:, wc])
                    wq_cs.append(wq_c)
                ktd_sb = kp.tile([64 + KB, KV, KB, 128], BF16)
                nc.sync.dma_start(ktd_sb[:], kT[:])
                va0_sb = kp.tile([128, KV, KB, 128], BF16)
                nc.sync.dma_start(va0_sb[:], vaug0[:])
                va1_sb = kp.tile([128, KV, KB, 128], BF16)
                nc.sync.dma_start(va1_sb[:], vaug1[:])
                tri_sb = kp.tile([128, 128], BF16)
                nc.sync.dma_start(tri_sb[:], tri[:])
                ebc_sb = kp.tile([2, 128], F32R)
                nc.sync.dma_start(ebc_sb[:], ebc[:])

                for m in range(NP):
                    wq_c = wq_cs[m // 4]
                    me = 128 * (m % 4)
                    qp = psb.tile([128, CH], F32, tag="big")
                    for kt in range(DT):
                        for n0, nn in c["NC"]:
                            nc.tensor.matmul(
                                qp[:, n0:n0 + nn],
                                wq_c[:, kt, me:me + 128],
                                xts[:, kt, n0:n0 + nn],
                                start=(kt == 0), stop=(kt == DT - 1))
                    qcp = rp.tile([128, CH], F32, tag="qcp")
                    nc.scalar.copy(qcp[:], qp[:])
                    qs = rp.tile([128, CH], F32, tag="qs")
                    nc.vector.stream_shuffle(qs[:], qcp[:], swap)
                    t1 = rp.tile([128, CH], F32, tag="t1")
                    nc.vector.tensor_mul(t1[:], qcp[:], cos_sb[:])
                    t2 = rp.tile([128, CH], F32, tag="t2")
                    nc.gpsimd.tensor_mul(t2[:], qs[:], sin_sb[:])
                    qt = qtp.tile([128, CH], BF16, tag="qt", bufs=2)
                    nc.gpsimd.tensor_add(qt[:], t1[:], t2[:])
                    if dbg and m == 0:
                        nc.sync.dma_start(qdbg[:], qt[:])
                    qts.append(qt)
                    # per-head q tiles with the causal-bias rows appended:
                    # contraction rows 0:64 = q, 64:72 = brow bias rows
                    qh0 = qtp.tile([64 + KB, CH], BF16, tag="qh0", bufs=2)
                    nc.sync.dma_start(qh0[0:64], qt[0:64])
                    nc.sync.dma_start(qh0[64:64 + KB], brow[:])
                    qh1 = qtp.tile([64 + KB, CH], BF16, tag="qh1", bufs=2)
                    nc.sync.dma_start(qh1[0:64], qt[64:128])
                    nc.sync.dma_start(qh1[64:64 + KB], brow[:])
                    qhs = [qh0, qh1]

                    kv = (2 * m) // hpkv
                    av0 = psa.tile([128, CH], F32, tag="av0")
                    av1 = psa.tile([128, CH], F32, tag="av1")
                    for kb in range(KB):
                        # QK with the causal bias folded into the K=72
                        # contraction; both heads share one stationary
                        sps = []
                        for hh in range(2):
                            sp = psb.tile([128, CH], F32, tag="big")
                            lh = ktd_sb[:, kv, kb, :]
                            rh = qhs[hh]
                            for n0, nn in c["NC"]:
                                nc.tensor.matmul(
                                    sp[:, n0:n0 + nn], lh,
                                    rh[:, n0:n0 + nn],
                                    start=True, stop=True)
                            sps.append(sp)
                        ers = []
                        for hh in range(2):
                            er = ep.tile([128, CH], BF16, tag="er")
                            nc.scalar.activation(er[:], sps[hh][:], AF.Exp)
                            if 128 * (kb + 1) <= CH:
                                dsl = slice(128 * kb, 128 * (kb + 1))
                                nc.vector.tensor_mul(er[:, dsl], er[:, dsl],
                                                     tri_sb[:])
                            if dbg and m == 0 and kb == 0:
                                nc.sync.dma_start(
                                    (edbg if hh == 0 else e1dbg)[:], er[:])
                            ers.append(er)
                        for n0, nn in c["NC"]:
                            nc.tensor.matmul(
                                av0[:, n0:n0 + nn],
                                va0_sb[:, kv, kb, :], ers[0][:, n0:n0 + nn],
                                start=(kb == 0), stop=(kb == KB - 1))
                        for n0, nn in c["NC"]:
                            nc.tensor.matmul(
                                av1[:, n0:n0 + nn],
                                va1_sb[:, kv, kb, :], ers[1][:, n0:n0 + nn],
                                start=(kb == 0), stop=(kb == KB - 1))
                    # normalize: den_h0 at av0 row 64, den_h1 at av1 row 0.
                    # Stage V-rows + dens out of PSUM, broadcast raw dens
                    # via two K=1 matmuls into the freed av0 ring slot,
                    # then ONE fast-approx reciprocal over the full tile
                    # (vector.reciprocal costs 6.5us/call on HW; the
                    # custom fast op is only safe at base partition 0).
                    au = rcp.tile([128, CH], F32R, tag="au")
                    nc.vector.tensor_copy(au[0:64], av0[0:64])
                    nc.vector.tensor_copy(au[64:128], av1[64:128])
                    stg = rcp.tile([128, CH], F32R, tag="stg", bufs=1)
                    nc.scalar.copy(stg[64:65], av0[64:65])
                    nc.scalar.copy(stg[0:1], av1[0:1])
                    den2 = rcp.tile([2, CH], F32R, tag="den2", bufs=1)
                    nc.sync.dma_start(den2[0:1], stg[64:65])
                    nc.sync.dma_start(den2[1:2], stg[0:1])
                    bcd = psa.tile([128, CH], F32, tag="av0")
                    for n0, nn in c["NC"]:
                        nc.tensor.matmul(bcd[:, n0:n0 + nn], ebc_sb[:],
                                         den2[:, n0:n0 + nn],
                                         start=True, stop=True)
                    bcs = rcp.tile([128, CH], F32, tag="bcs", bufs=1)
                    nc.vector.reciprocal_approx_fast(bcs[:], bcd[:])
                    at = atp.tile([128, CH], BF16, tag=f"at{m}")
                    nc.vector.tensor_mul(at[:], au[:], bcs[:])
                    if dbg and m == 0:
                        nc.sync.dma_start(rdbg[0:1], bcs[0:1])
                        nc.sync.dma_start(rdbg[1:2], bcs[127:128])
                        nc.sync.dma_start(adbg[:], at[:])
                    ats.append(at)

            # ---- Phase C: out[q, o] = sum_m at_m.T @ woT_m
            with (
                tc.tile_pool(name="wo", bufs=2) as wop,
                tc.tile_pool(name="osb", bufs=3) as op_,
            ):
                MQ = CH // 128
                for oc, (o0, ow) in enumerate(c["OC"]):
                    wo_c = wop.tile([128, NP, 512], BF16, tag="wo")
                    nc.sync.dma_start(wo_c[:], wor[:, oc])
                    for mq in range(MQ):
                        opx = psb.tile([128, 512], F32, tag="big")
                        for kq in range(NP):
                            nc.tensor.matmul(
                                opx[:, :ow],
                                ats[kq][:, 128 * mq:128 * (mq + 1)],
                                wo_c[:, kq, :ow],
                                start=(kq == 0), stop=(kq == NP - 1))
                        osb = op_.tile([128, 512], F32, tag="os")
                        if mq % 2 == 0:
                            nc.scalar.copy(osb[:, :ow], opx[:, :ow])
                        else:
                            nc.vector.tensor_copy(osb[:, :ow], opx[:, :ow])
                        nc.sync.dma_start(
                            out[128 * mq:128 * (mq + 1), o0:o0 + ow],
                            osb[:, :ow])
    nc.compile()
    return nc


def host_inputs(cfg, x, k_cache, v_cache, Wq, Wo, core):
    import ml_dtypes
    c = _derived(cfg)
    CH, KB, KV, W, DH = c["CH"], c["KB"], c["KV"], c["W"], c["DH"]
    b, ch = core // 4, core % 4
    Tc = k_cache.shape[2]
    f32 = np.float32
    bf16 = ml_dtypes.bfloat16

    DT, NP, D = c["DT"], c["NP"], c["D"]
    xT = x[b, CH * ch:CH * (ch + 1), :].T            # (D, CH)
    xr = np.ascontiguousarray(
        xT.reshape(DT, 128, CH).transpose(1, 0, 2)).astype(bf16)
    WC = c["WC"]
    wqT = Wq.T * f32(1.0 / np.sqrt(DH))              # (D, D)
    wqr = np.ascontiguousarray(
        wqT.reshape(DT, 128, WC, 512).transpose(1, 2, 0, 3)).astype(bf16)
    woT = Wo.T                                       # (D, D)
    wor = np.ascontiguousarray(
        woT.reshape(NP, 128, WC, 512).transpose(1, 2, 0, 3)).astype(bf16)
    kw = k_cache[b, :, Tc - W:, :]                      # (KV, W, DH)
    kT = np.zeros((64 + KB, KV, KB, 128), f32)
    kT[0:64] = kw.reshape(KV, KB, 128, DH).transpose(3, 0, 1, 2)
    for r in range(KB):
        kT[64 + r, :, r, :] = 1.0
    kT = kT.astype(bf16)
    vw = v_cache[b, :, Tc - W:, :].reshape(KV, KB, 128, DH)
    vaug0 = np.zeros((128, KV, KB, 128), f32)
    vaug0[:, :, :, :DH] = vw.transpose(2, 0, 1, 3)
    vaug0[:, :, :, DH] = 1.0
    vaug1 = np.zeros((128, KV, KB, 128), f32)
    vaug1[:, :, :, 64:128] = vw.transpose(2, 0, 1, 3)
    vaug1[:, :, :, 0] = 1.0
    pos = (CH * ch + np.arange(CH)).astype(f32)
    inv = 1.0 / (cfg["BASE"] ** (np.arange(0, DH, 2, dtype=f32) / DH))
    r = np.arange(128)
    u = (r % 64) // 2
    ang = pos[None, :] * inv[u][:, None]                # (128, CH)
    cosT = np.cos(ang).astype(bf16)
    sinT = (np.sin(ang) * np.where(r % 2 == 0, -1.0, 1.0)[:, None]).astype(bf16)
    if ch == 0:
        tri = (np.arange(128)[:, None] <= np.arange(128)[None, :]).astype(f32)
        brow = np.zeros((KB, CH), f32)
        for kb in range(KB):
            brow[kb, :128 * kb] = BIGNEG
    else:
        tri = np.ones((128, 128), f32)
        brow = np.zeros((KB, CH), f32)
    ebc = np.zeros((2, 128), f32)
    ebc[0, 0:64] = 1.0
    ebc[1, 64:128] = 1.0
    return {"xr": xr, "wqr": wqr, "wor": wor, "kT": kT,
            "vaug0": vaug0.astype(bf16), "vaug1": vaug1.astype(bf16),
            "cosT": cosT, "sinT": sinT, "tri": tri.astype(bf16),
            "brow": brow.astype(bf16), "ebc": ebc}


_NC_CACHE = {}


def run(cfg, x, k_cache, v_cache, Wq, Wo, trace=False):
    key = tuple(sorted((k, v) for k, v in cfg.items()))
    if key not in _NC_CACHE:
        _NC_CACHE[key] = build(cfg)
    nc = _NC_CACHE[key]
    in_maps = [host_inputs(cfg, x, k_cache, v_cache, Wq, Wo, c)
               for c in range(8)]
    res = None
    for attempt in range(3):
        try:
            res = run_bass_kernel_spmd(nc, in_maps, core_ids=list(range(8)),
                                       trace=trace)
            break
        except Exception:
            if attempt == 2:
                raise
            time.sleep(2.0)
    outs = [res.results[c]["out"] for c in range(8)]
    full = np.stack([np.concatenate(outs[0:4], axis=0),
                     np.concatenate(outs[4:8], axis=0)])
    return full, res


def kernel(x, k_cache, v_cache, Wq, Wo):
    full, _ = run(FULL, np.asarray(x), np.asarray(k_cache),
                  np.asarray(v_cache), np.asarray(Wq), np.asarray(Wo))
    return full.astype(np.float32)
